# revision 40
# baseline (speedup 1.0000x reference)
"""DialogueGCN Trainium2 kernel — 8-core SPMD row-sharded implementation.

Decomposition (validated in numpy):
  attn = softmax(band(x@x.T)) has off-band entries equal to a per-row constant
  c_i = exp(-m_i)/Z_i.  Each relation adjacency adj_k = mask_k * attn splits into
    adj_k @ s = [A_k^ext @ s_ext]   (per-96-row-block: c_i*mask within own block
                                     + band corrections over +-10 cols)
    + c_i * (E_rows @ H_k)          (cross-block per-speaker-class prefix/suffix
                                     sums of s, via a tiny AllGather of per-block
                                     class sums G)
  Mini-blocks (10 halo rows each side) replicate neighbour-core h1 rows locally
  so layer 2 needs no halo exchange.
"""
import os
import sys

for _p in ("/opt/trn_rl_repo", "/root/.axon_site/_ro/trn_rl_repo"):
    if os.path.isdir(_p) and _p not in sys.path:
        sys.path.insert(0, _p)

import numpy as np
import ml_dtypes

import concourse.bass as bass
import concourse.mybir as mybir
import concourse.tile as tile
from concourse import masks
from concourse.bass_utils import run_bass_kernel_spmd

N, D, WIN, NSPK, NEMO = 6144, 128, 10, 8, 7
NSPK1 = NSPK + 1           # 8 speaker classes + all-ones "block total" class
E4R = 4 * NSPK + 2         # e4 rows: 4 rels x 8 classes + totS + totP
CORES, R, B, NBL = 8, 768, 96, 8
EXT = B + 2 * WIN          # 116
HALO = B + WIN             # 106
XR = R + 2 * HALO          # 980
NBG = CORES * NBL          # 64
F32 = mybir.dt.float32
BF16 = mybir.dt.bfloat16
AOT = mybir.AluOpType
ACTF = mybir.ActivationFunctionType

# block geometry: (t, ostart, P, estart, mini_col)  in local l coords
FULL_TS = [(t, HALO + B * t, B, B + B * t, None) for t in range(NBL)]
MINI_TS = [(8, B, WIN, 0, 0), (9, HALO + R, WIN, HALO + R - WIN - B, 1)]
# mini R: rows l in [874, 884), ext cols [864, 980) -> estart = 864 = HALO+R-WIN-B? 106+768-10-96=768? no:
MINI_TS = [(8, B, WIN, 0, 0), (9, HALO + R, WIN, XR - EXT, 1)]


def _bcast(ap, shape):
    return ap.broadcast_to(shape)


def build_program():
    nc = bass.Bass()
    dp = nc.declare_dram_parameter

    xT_d = dp("xT", [D, XR], F32, isOutput=False)
    eT_d = dp("eT", [NSPK, XR], BF16, isOutput=False)
    eO_d = dp("eO", [NBL * EXT, NSPK1], BF16, isOutput=False)
    e4T_d = dp("e4T", [E4R, R], BF16, isOutput=False)
    e4Tm_d = dp("e4Tm", [E4R, 2 * WIN], BF16, isOutput=False)
    w41_d = dp("w41", [D, 4 * D], BF16, isOutput=False)
    w42_d = dp("w42", [D, 4 * D], BF16, isOutput=False)
    wag1_d = dp("wag1", [D, D], BF16, isOutput=False)
    wag2_d = dp("wag2", [D, D], BF16, isOutput=False)
    we1_d = dp("we1", [2 * D, D], BF16, isOutput=False)
    we2_d = dp("we2", [D, NEMO], BF16, isOutput=False)
    ws_d = dp("ws", [2 * D, NEMO], BF16, isOutput=False)
    be1_d = dp("be1", [D, 1], F32, isOutput=False)
    be2_d = dp("be2", [NEMO, 1], F32, isOutput=False)
    bs_d = dp("bs", [NEMO, 1], F32, isOutput=False)
    # shape constants: single-block [B, EXT], block-tiled [B, NBL*EXT],
    # mini variants [WIN, 2, EXT]
    cnames = ["band", "pred", "suc", "predib", "sucib", "diagm"]
    c_d = {n: dp("c_" + n, [B, EXT], F32, isOutput=False)
           for n in ("band", "predib", "sucib")}
    c8_d = {n: dp("c8_" + n, [B, NBL * EXT], F32, isOutput=False)
            for n in ("pred", "suc", "diagm")}
    cm_d = {n: dp("cm_" + n, [WIN, 2, EXT], F32, isOutput=False) for n in cnames}
    triL_d = dp("triL", [NBL, 2, 10], BF16, isOutput=False)
    triR_d = dp("triR", [3 * CORES, 2, 10], BF16, isOutput=False)
    vmask_d = dp("vmask", [WIN, 2], F32, isOutput=False)
    emo_d = dp("emo", [NEMO, R], F32, isOutput=True)
    sen_d = dp("sen", [NEMO, R], F32, isOutput=True)

    # AllGather payload: 3 slots per core = [core-sum, first block G,
    # last block G]; per-block G stays in a LOCAL dram buffer.
    ag_in = [nc.dram_tensor(f"ag{L}_in", [3, NSPK1, 4 * D], BF16) for L in (1, 2)]
    ag_out = [
        nc.dram_tensor(f"ag{L}_out", [3 * CORES, NSPK1, 4 * D], BF16,
                       addr_space="Shared")
        for L in (1, 2)
    ]
    gloc_d = [nc.dram_tensor(f"gloc{L}", [NBL, NSPK1, 4 * D], BF16)
              for L in (1, 2)]

    with tile.TileContext(nc) as tc:
        with tc.tile_pool(name="persist", bufs=1) as pp, \
             tc.tile_pool(name="cpool", bufs=1) as cp:
            # ---- load inputs / constants ----
            # distribute input-load DMA triggers across engine queues: the
            # SP queue pays ~620ns per trigger, so serializing ~30 loads
            # there delays everything queued behind them.
            _eng_rr = [nc.sync, nc.gpsimd, nc.scalar, nc.sync, nc.gpsimd]
            _eng_i = [0]

            def _load(out, in_):
                _eng_rr[_eng_i[0] % len(_eng_rr)].dma_start(out=out, in_=in_)
                _eng_i[0] += 1

            xT = pp.tile([D, XR], F32)
            nc.sync.dma_start(out=xT[:, 0:490], in_=xT_d[:, 0:490])
            nc.scalar.dma_start(out=xT[:, 490:XR], in_=xT_d[:, 490:XR])
            xTb = pp.tile([D, XR], BF16)
            nc.vector.tensor_copy(xTb[:], xT[:])
            eT = pp.tile([NSPK, XR], BF16)
            _load(eT[:], eT_d[:])
            eO = pp.tile([EXT, NBL, NSPK1], BF16)
            _load(eO[:], eO_d[:].rearrange("(b p) c -> p b c", p=EXT))
            e4T = pp.tile([E4R, R], BF16)
            _load(e4T[:], e4T_d[:])
            e4Tm = pp.tile([E4R, 2 * WIN], BF16)
            _load(e4Tm[:], e4Tm_d[:])
            w41 = pp.tile([D, 4 * D], BF16)
            nc.gpsimd.dma_start(out=w41[:], in_=w41_d[:])
            w42 = pp.tile([D, 4 * D], BF16)
            _load(w42[:], w42_d[:])
            wag1 = pp.tile([D, D], BF16)
            _load(wag1[:], wag1_d[:])
            wag2 = pp.tile([D, D], BF16)
            _load(wag2[:], wag2_d[:])
            we1a = pp.tile([D, D], BF16)
            _load(we1a[:], we1_d[0:D, :])
            we1b = pp.tile([D, D], BF16)
            _load(we1b[:], we1_d[D:2 * D, :])
            we2 = pp.tile([D, NEMO], BF16)
            _load(we2[:], we2_d[:])
            wsa = pp.tile([D, NEMO], BF16)
            _load(wsa[:], ws_d[0:D, :])
            wsb = pp.tile([D, NEMO], BF16)
            _load(wsb[:], ws_d[D:2 * D, :])
            be1 = pp.tile([D, 1], F32)
            _load(be1[:], be1_d[:])
            be2 = pp.tile([NEMO, 1], F32)
            _load(be2[:], be2_d[:])
            bs = pp.tile([NEMO, 1], F32)
            _load(bs[:], bs_d[:])
            cst = {}
            for n in ("band", "predib", "sucib"):
                cst[n] = cp.tile([B, EXT], F32, name="c_" + n)
                _load(cst[n][:], c_d[n][:])
            cst8 = {}
            for n in ("pred", "suc", "diagm"):
                cst8[n] = cp.tile([B, NBL, EXT], F32, name="c8_" + n)
                _load(cst8[n][:],
                      c8_d[n][:].rearrange("p (b e) -> p b e", e=EXT))
            cstm = {}
            for n in cnames:
                cstm[n] = cp.tile([WIN, 2, EXT], F32, name="cm_" + n)
                _load(cstm[n][:], cm_d[n][:])
            triLS = pp.tile([NBL, 10], BF16)
            _load(triLS[:], triL_d[:, 0, :])
            triLP = pp.tile([NBL, 10], BF16)
            _load(triLP[:], triL_d[:, 1, :])
            triRS = pp.tile([3 * CORES, 10], BF16)
            _load(triRS[:], triR_d[:, 0, :])
            triRP = pp.tile([3 * CORES, 10], BF16)
            _load(triRP[:], triR_d[:, 1, :])
            vmask = pp.tile([WIN, 2], F32)
            _load(vmask[:], vmask_d[:])
            idf = pp.tile([128, 128], F32)
            masks.make_identity(nc, idf[:])
            idb = pp.tile([128, 128], BF16)
            masks.make_identity(nc, idb[:])

            # ---- persistent state tiles ----
            h1T = pp.tile([D, R + 2 * WIN], BF16)       # col = l - 96
            h2T = pp.tile([D, R], BF16)
            cB = pp.tile([B, NBL], F32)
            dB = pp.tile([B, NBL], F32)
            cM = pp.tile([WIN, 2], F32)
            dM = pp.tile([WIN, 2], F32)
            # A^T batch tiles: [EXT, nb, P] per relation; AT[(k, t)] are APs
            ATbF = {k: pp.tile([EXT, NBL, B], BF16, name=f"ATF{k}")
                    for k in range(4)}
            ATbM = {k: pp.tile([EXT, 2, WIN], BF16, name=f"ATM{k}")
                    for k in range(4)}
            AT = {}
            for t, _, P, _, _ in FULL_TS:
                for k in range(4):
                    AT[(k, t)] = ATbF[k][:, t, :]
            for t, _, P, _, _ in MINI_TS:
                for k in range(4):
                    AT[(k, t)] = ATbM[k][:, t - 8, :]
            accM = {}
            accA = {}
            for t, _, P, _, _ in FULL_TS + MINI_TS:
                accM[(t, 1)] = pp.tile([P, D], F32, name=f"accM1_{t}")
                accA[(t, 1)] = pp.tile([P, D], F32, name=f"accA1_{t}")
                if t < NBL:
                    accM[(t, 2)] = pp.tile([P, D], F32, name=f"accM2_{t}")
                    accA[(t, 2)] = pp.tile([P, D], F32, name=f"accA2_{t}")

            # ---------- helpers ----------
            SPL = 6      # elementwise split: blocks [0:SPL] on DVE, rest GpSimd

            def split_tt(out, in0, in1, op, nb):
                """emit a batched [P, nb, EXT] tensor_tensor split DVE/GpSimd"""
                if nb <= 2 or SPL >= nb:
                    nc.vector.tensor_tensor(out, in0, in1, op)
                    return
                nc.vector.tensor_tensor(
                    out[:, 0:SPL, :], in0[:, 0:SPL, :], in1[:, 0:SPL, :], op)
                nc.gpsimd.tensor_tensor(
                    out[:, SPL:nb, :], in0[:, SPL:nb, :], in1[:, SPL:nb, :], op)

            # =============== layer part 1: s, G, AllGather (+ scores L1) =======
            # The G-chain (per-block class sums -> AllGather input) is emitted
            # FIRST under high_priority so the collective triggers as early as
            # its data allows and overlaps with the attention math, instead of
            # queueing behind it.
            def layer_part1(L, hT, hoff, w4, agi, ago, gloc, sp, psp, psg, pss_p,
                            gp, ts_list, score_sink=None):
                s_tiles = {}
                dmae = nc.gpsimd if L == 1 else nc.sync
                with tc.high_priority():
                    if L == 1:
                        psSum = pss_p.tile([NSPK1, 4 * D], F32, name=f"psS{L}")
                    else:
                        # L2 runs inside the PE-saturated window: accumulate
                        # the core-sum on the DVE instead (it has slack there)
                        accS = gp.tile([NSPK1, 4 * D], F32, name="accS2")
                    for t, ostart, P, estart, _ in ts_list:
                        if t >= NBL:
                            continue
                        pss = psp.tile([EXT, 4 * D], F32, name=f"pss{L}",
                                       tag="pss")
                        nc.tensor.matmul(
                            pss[:], hT[:, estart - hoff:estart - hoff + EXT],
                            w4[:], start=True, stop=True)
                        sAll = sp.tile([EXT, 4 * D], BF16, name=f"sAll{L}_{t}")
                        (nc.scalar.copy if t % 2 else nc.vector.tensor_copy)(
                            sAll[:], pss[:])
                        s_tiles[t] = sAll
                        ps2 = psg.tile([NSPK1, 4 * D], F32, name=f"psg{L}",
                                       tag="psg")
                        nc.tensor.matmul(
                            ps2[:], eO[:, t, :], sAll[:], start=True, stop=True)
                        if L == 1:
                            # accumulate core-sum (one PSUM group over blocks)
                            nc.tensor.matmul(
                                psSum[:], eO[:, t, :], sAll[:],
                                start=(t == 0), stop=(t == NBL - 1))
                        gsb = gp.tile([NSPK1, 4 * D], BF16, name=f"gsb{L}",
                                      tag="gsb")
                        (nc.vector.tensor_copy if t % 2 else nc.scalar.copy)(
                            gsb[:], ps2[:])
                        if L == 2:
                            if t == 0:
                                nc.vector.tensor_copy(accS[:], ps2[:])
                            else:
                                nc.vector.tensor_tensor(
                                    accS[:], accS[:], ps2[:], AOT.add)
                        dmae.dma_start(out=gloc[t], in_=gsb[:])
                        if t == 0:
                            dmae.dma_start(out=agi[1], in_=gsb[:])
                        elif t == NBL - 1:
                            dmae.dma_start(out=agi[2], in_=gsb[:])
                    gsum = gp.tile([NSPK1, 4 * D], BF16, name=f"gsum{L}",
                                   tag="gsum")
                    if L == 1:
                        nc.scalar.copy(gsum[:], psSum[:])
                    else:
                        nc.scalar.copy(gsum[:], accS[:])
                    dmae.dma_start(out=agi[0], in_=gsum[:])
                    nc.gpsimd.collective_compute(
                        "AllGather", AOT.bypass,
                        replica_groups=[list(range(CORES))],
                        ins=[agi[:]], outs=[ago[:]],
                    )
                for i, (t, ostart, P, estart, _) in enumerate(ts_list):
                    if t >= NBL:
                        pss = psp.tile([EXT, 4 * D], F32, name=f"pss{L}",
                                       tag="pss")
                        nc.tensor.matmul(
                            pss[:], hT[:, estart - hoff:estart - hoff + EXT],
                            w4[:], start=True, stop=True)
                        sAll = sp.tile([EXT, 4 * D], BF16, name=f"sAll{L}_{t}")
                        if i % 2 == 0:
                            nc.vector.tensor_copy(sAll[:], pss[:])
                        else:
                            nc.scalar.copy(sAll[:], pss[:])
                        s_tiles[t] = sAll
                    pag = psg.tile([B, D], F32, name=f"pag{L}", tag="pag")
                    nc.tensor.matmul(
                        pag[:P, :], hT[:, ostart - hoff:ostart - hoff + P],
                        (wag1 if L == 1 else wag2)[:], start=True, stop=True)
                    nc.vector.tensor_copy(accA[(t, L)][:], pag[:P, :])
                    if score_sink is not None:
                        score_sink(t, ostart, P, estart)
                return s_tiles

            # =============== attention math (layer independent) ===============
            def a_build(ab, ps_tr, blocks, PP, nb, cd, sb, smT, c_out, d_out,
                        tag, ATt):
                """sb: [PP, nb, EXT] banded scores; smT: [EXT, nb, PP]
                transposed same-speaker masks (bf16). cd: 'predib','sucib' ->
                per-block [P,EXT] AP fns; 'pred3','suc3','diagm3' ->
                [PP, nb, EXT] real-tile APs. ATt: k -> [EXT, nb, PP] output."""
                sh3 = [PP, nb, EXT]
                mB = ab.tile([PP, nb], F32, name=f"mB{tag}")       # holds -m
                nc.vector.tensor_reduce(
                    mB[:], sb[:], axis=mybir.AxisListType.X, op=AOT.max,
                    negate=True)
                exv = ab.tile(sh3, F32, name=f"exv{tag}")
                sumB = ab.tile([PP, nb], F32, name=f"sumB{tag}")
                for j in range(nb):
                    # exp(s - m) in one ACT op: out = Exp(in + bias), bias = -m
                    nc.scalar.activation(
                        exv[:, j, :], sb[:, j, :], ACTF.Exp,
                        bias=mB[:, j:j + 1], accum_out=sumB[:, j:j + 1])
                enB = ab.tile([PP, nb], F32, name=f"enB{tag}")
                nc.scalar.activation(enB[:], mB[:], ACTF.Exp)
                ZB = ab.tile([PP, nb], F32, name=f"ZB{tag}")
                nc.vector.scalar_tensor_tensor(
                    ZB[:], enB[:], float(N - EXT), sumB[:], AOT.mult, AOT.add)
                rZ = ab.tile([PP, nb], F32, name=f"rZ{tag}")
                nc.vector.reciprocal(rZ[:], ZB[:])
                nc.vector.tensor_tensor(c_out, enB[:], rZ[:], AOT.mult)
                dg = ab.tile(sh3, F32, name=f"dg{tag}")
                split_tt(dg[:], exv[:], cd["diagm3"], AOT.mult, nb)
                d0 = ab.tile([PP, nb], F32, name=f"d0{tag}")
                nc.vector.tensor_reduce(
                    d0[:], dg[:], axis=mybir.AxisListType.X, op=AOT.add)
                nc.vector.tensor_tensor(d_out, d0[:], rZ[:], AOT.mult)
                u = ab.tile(sh3, F32, name=f"u{tag}")
                for j in range(nb):
                    nc.vector.tensor_scalar(
                        u[:, j, :], exv[:, j, :], enB[:, j:j + 1], rZ[:, j:j + 1],
                        AOT.subtract, AOT.mult)
                up = ab.tile(sh3, F32, name=f"up{tag}")
                split_tt(up[:], u[:], cd["pred3"], AOT.mult, nb)
                un = ab.tile(sh3, F32, name=f"un{tag}")
                split_tt(un[:], u[:], cd["suc3"], AOT.mult, nb)
                # w1/w2 in bf16: sm is exactly 0/1, so rounding w before the
                # mask multiply gives bit-identical A_k to rounding after.
                w1 = ab.tile(sh3, BF16, name=f"w1{tag}")
                w2 = ab.tile(sh3, BF16, name=f"w2{tag}")
                for j in range(nb):
                    nc.vector.scalar_tensor_tensor(
                        w1[:, j, :], cd["predib"](j), c_out[:, j:j + 1],
                        up[:, j, :], AOT.mult, AOT.add)
                    nc.vector.scalar_tensor_tensor(
                        w2[:, j, :], cd["sucib"](j), c_out[:, j:j + 1],
                        un[:, j, :], AOT.mult, AOT.add)
                # transpose w1/w2 per block, then do the same/diff mask algebra
                # in transposed bf16 layout (A3 = w1 - A1, A4 = w2 - A2)
                w1T = ab.tile([EXT, nb, PP], BF16, name=f"w1T{tag}")
                w2T = ab.tile([EXT, nb, PP], BF16, name=f"w2T{tag}")
                for j, (t, ostart, P, estart, _) in enumerate(blocks):
                    for src, dstT in ((w1, w1T), (w2, w2T)):
                        pst = ps_tr.tile([EXT, PP], BF16, name="pst", tag="pst")
                        nc.tensor.matmul(
                            pst[:, :P], src[:P, j, :], idb[:P, :P],
                            is_transpose=True, start=True, stop=True)
                        nc.any.tensor_copy(dstT[:, j, :P], pst[:, :P])
                nc.vector.tensor_tensor(ATt[0][:], smT[:], w1T[:], AOT.mult)
                nc.vector.tensor_tensor(ATt[1][:], smT[:], w2T[:], AOT.mult)
                nc.vector.tensor_tensor(ATt[2][:], w1T[:], ATt[0][:],
                                        AOT.subtract)
                nc.vector.tensor_tensor(ATt[3][:], w2T[:], ATt[1][:],
                                        AOT.subtract)

            def part2_order(ts_list):
                if len(ts_list) <= NBL:
                    return ts_list
                by_t = {t[0]: t for t in ts_list}
                order = [8, 0, 1, 2, 3, 4, 5, 6, 9, 7]
                return [by_t[t] for t in order]

            # =============== layer part 2: A-matmuls, H, cross, combine ========
            def layer_part2(L, hT, hoff, ago, gloc, s_tiles, ts_list):
                ts_list = part2_order(ts_list)
                with tc.tile_pool(name=f"psA{L}", bufs=3, space="PSUM") as psa:
                    for t, ostart, P, estart, mcol in ts_list:
                        pm = psa.tile([P, D], F32, name=f"pm{L}", tag="pm")
                        for k in range(4):
                            nc.tensor.matmul(
                                pm[:], AT[(k, t)],
                                s_tiles[t][:, k * D:(k + 1) * D],
                                start=(k == 0), stop=(k == 3))
                        dsl = (dB[:, t:t + 1] if t < NBL
                               else dM[:, mcol:mcol + 1])
                        # accC = aggr*d + sum_k A_k @ s_k
                        nc.vector.scalar_tensor_tensor(
                            accM[(t, L)][:], accA[(t, L)][:], dsl, pm[:],
                            AOT.mult, AOT.add)
                with tc.tile_pool(name=f"hL{L}", bufs=1) as hp:
                    gf = hp.tile([3 * CORES, NSPK1, 4, D], BF16, name=f"gf{L}")
                    ago_v = ago[:].rearrange("g c (r d) -> g c r d", r=4)
                    # local per-block G (no collective dependency): the local
                    # half of every H prefix-sum can run during the AllGather
                    gl = hp.tile([NBL, NSPK1, 4, D], BF16, name=f"gl{L}")
                    nc.scalar.dma_start(
                        out=gl[:],
                        in_=gloc[:].rearrange("g c (r d) -> g c r d", r=4))
                    # fence: gf loads (and so every gf-dependent matmul) only
                    # become schedulable after the last A-matmul combine, so
                    # the PE queue keeps all overlap work AHEAD of the
                    # collective-gated instructions (avoids head-of-line
                    # blocking during the AllGather).
                    fence = hp.tile([1, 1], F32, name=f"fence{L}")
                    nc.gpsimd.tensor_copy(
                        fence[:], accM[(ts_list[-1][0], L)][0:1, 0:1])
                    nc.gpsimd.dma_start(out=gf[:], in_=ago_v[:])
                    # hcat slots: [rel*8+c] raw per-class H, 32/33 = block
                    # totals (tot - H_c is folded into the pc contraction via
                    # negated e4 rows + ones rows for the tot slots).
                    hcat = hp.tile([10, E4R, D], BF16, name=f"hcat{L}")
                    h_srcs = [
                        (0, triLS, triRS, 0),   # k=1 same-pred
                        (1, triLP, triRP, 1),   # k=2 same-suc
                        (2, triLS, triRS, 2),   # k=3 diff-pred
                        (3, triLP, triRP, 3),   # k=4 diff-suc
                    ]
                    with tc.tile_pool(name=f"psH{L}", bufs=6, space="PSUM") as psh:
                        for rel, tl, tr, rr in h_srcs:
                            for c0 in (0, 4):
                                ph = psh.tile([10, 4 * D], F32, name=f"ph{L}",
                                              tag="ph")
                                nc.tensor.matmul(
                                    ph[:], tl[:], gl[:, c0:c0 + 4, rr, :],
                                    start=True, stop=False)
                                nc.tensor.matmul(
                                    ph[:], tr[:], gf[:, c0:c0 + 4, rr, :],
                                    start=False, stop=True)
                                s0 = rel * NSPK + c0
                                (nc.vector.tensor_copy if c0 else nc.scalar.copy)(
                                    hcat[:, s0:s0 + 4, :],
                                    ph[:].rearrange("p (c d) -> p c d", d=D))
                        pt = psh.tile([10, 2 * D], F32, name=f"pt{L}", tag="ph")
                        nc.tensor.matmul(pt[:, 0:D], triLS[:],
                                         gl[:, NSPK, 2, :],
                                         start=True, stop=False)
                        nc.tensor.matmul(pt[:, 0:D], triRS[:],
                                         gf[:, NSPK, 2, :],
                                         start=False, stop=True)
                        nc.tensor.matmul(pt[:, D:2 * D], triLP[:],
                                         gl[:, NSPK, 3, :],
                                         start=True, stop=False)
                        nc.tensor.matmul(pt[:, D:2 * D], triRP[:],
                                         gf[:, NSPK, 3, :],
                                         start=False, stop=True)
                        nc.scalar.copy(
                            hcat[:, 4 * NSPK:E4R, :],
                            pt[:].rearrange("p (c d) -> p c d", d=D))
                    with tc.tile_pool(name=f"xb{L}", bufs=1) as xb, \
                         tc.tile_pool(name=f"psX{L}", bufs=2, space="PSUM") as psx:
                        hm4s = {}
                        _dq = [nc.sync, nc.gpsimd, nc.scalar]
                        for di, (t, ostart, P, estart, mcol) in enumerate(ts_list):
                            hm4 = xb.tile([E4R, D], BF16, name=f"hm4{L}_{t}")
                            _dq[di % 3].dma_start(
                                out=hm4[:], in_=hcat[t:t + 1, :, :])
                            hm4s[t] = hm4
                        for t, ostart, P, estart, mcol in ts_list:
                            pc = psx.tile([P, D], F32, name=f"pc{L}", tag="pc",
                                          bufs=4)
                            if t < NBL:
                                e4sl = e4T[:, B * t:B * t + P]
                            else:
                                e4sl = e4Tm[:, mcol * WIN:(mcol + 1) * WIN]
                            nc.tensor.matmul(
                                pc[:], e4sl, hm4s[t][:],
                                start=True, stop=True)
                            csl = (cB[:, t:t + 1] if t < NBL
                                   else cM[:, mcol:mcol + 1])
                            hrow = xb.tile([P, D], F32, name=f"hrow{L}",
                                           tag="hrow", bufs=4)
                            nc.vector.scalar_tensor_tensor(
                                hrow[:], pc[:], csl, accM[(t, L)][:],
                                AOT.mult, AOT.add)
                            if t >= NBL:
                                nc.vector.tensor_scalar_mul(
                                    hrow[:], hrow[:], vmask[:, mcol:mcol + 1])
                            ptr = psx.tile([D, P], F32, name=f"ptr{L}", tag="ptr",
                                           bufs=4)
                            nc.tensor.matmul(
                                ptr[:], hrow[:], idf[:P, :P],
                                is_transpose=True, start=True, stop=True)
                            if L == 1:
                                off = {8: 0, 9: R + WIN}.get(t, WIN + B * t)
                                nc.scalar.activation(
                                    h1T[:, off:off + P], ptr[:], ACTF.Relu)
                            else:
                                nc.scalar.activation(
                                    h2T[:, B * t:B * t + P], ptr[:], ACTF.Relu)

            # =============== head: two 384-wide chunks over h2T ===============
            def head():
                CH = 4 * B
                with tc.tile_pool(name="hd", bufs=2) as hd, \
                     tc.tile_pool(name="psE", bufs=2, space="PSUM") as pse:
                    for c0 in (0, CH):
                        h2c = h2T[:, c0:c0 + CH]
                        xc_ = xTb[:, HALO + c0:HALO + c0 + CH]
                        pe1 = pse.tile([D, CH], F32, name="pe1", tag="pe1")
                        nc.tensor.matmul(pe1[:], we1a[:], h2c,
                                         start=True, stop=False)
                        nc.tensor.matmul(pe1[:], we1b[:], xc_,
                                         start=False, stop=True)
                        e1b = hd.tile([D, CH], BF16, name="e1b", tag="e1b")
                        nc.scalar.activation(e1b[:], pe1[:], ACTF.Relu,
                                             bias=be1[:])
                        pe2 = pse.tile([NEMO, CH], F32, name="pe2", tag="pe2")
                        nc.tensor.matmul(pe2[:], we2[:], e1b[:],
                                         start=True, stop=True)
                        em1 = hd.tile([NEMO, CH], F32, name="em1", tag="em1")
                        nc.vector.tensor_scalar_add(em1[:], pe2[:], be2[:])
                        ps2 = pse.tile([NEMO, CH], F32, name="ps2", tag="pe2")
                        nc.tensor.matmul(ps2[:], wsa[:], h2c,
                                         start=True, stop=False)
                        nc.tensor.matmul(ps2[:], wsb[:], xc_,
                                         start=False, stop=True)
                        sn1 = hd.tile([NEMO, CH], F32, name="sn1", tag="em1")
                        nc.vector.tensor_scalar_add(sn1[:], ps2[:], bs[:])
                        # outputs stored transposed [NEMO, R]; host transposes
                        nc.sync.dma_start(out=emo_d[:, c0:c0 + CH], in_=em1[:])
                        nc.scalar.dma_start(out=sen_d[:, c0:c0 + CH], in_=sn1[:])

            # =============== orchestrate ===============
            L1_TS = FULL_TS + MINI_TS
            with tc.tile_pool(name="abuild", bufs=1) as ab:
                sbF = ab.tile([B, NBL, EXT], F32, name="sbF")
                smTF = ab.tile([EXT, NBL, B], BF16, name="smTF")
                sbM = ab.tile([WIN, 2, EXT], F32, name="sbM")
                smTM = ab.tile([EXT, 2, WIN], BF16, name="smTM")
                with tc.tile_pool(name="sL1", bufs=1) as sp1, \
                     tc.tile_pool(name="gL1", bufs=2) as gp1:
                    with tc.tile_pool(name="psL1", bufs=3, space="PSUM") as psp1, \
                         tc.tile_pool(name="psG1", bufs=1, space="PSUM") as psg1, \
                         tc.tile_pool(name="psS1", bufs=1, space="PSUM") as pss1, \
                         tc.tile_pool(name="ps_sc", bufs=1, space="PSUM") as ps_sc, \
                         tc.tile_pool(name="ps_sm", bufs=1, space="PSUM") as ps_sm:

                        def score_sink(t, ostart, P, estart):
                            j = t if t < NBL else t - NBL
                            sb_t = sbF if t < NBL else sbM
                            smT_t = smTF if t < NBL else smTM
                            bandap = (cst["band"][:] if t < NBL
                                      else cstm["band"][:, j, :])
                            pssc = ps_sc.tile([B, EXT], F32, name="pssc",
                                              tag="pssc")
                            nc.tensor.matmul(
                                pssc[:P, :], xT[:, ostart:ostart + P],
                                xT[:, estart:estart + EXT], start=True,
                                stop=True)
                            nc.vector.tensor_tensor(
                                sb_t[:P, j, :], pssc[:P, :], bandap[:P],
                                AOT.mult)
                            # speaker-same mask, produced directly transposed
                            pssm = ps_sm.tile([EXT, B], F32, name="pssm",
                                              tag="pssm")
                            nc.tensor.matmul(
                                pssm[:, :P], eT[:, estart:estart + EXT],
                                eT[:, ostart:ostart + P], start=True,
                                stop=True)
                            (nc.vector.tensor_copy if j % 2 else nc.scalar.copy)(
                                smT_t[:, j, :P], pssm[:, :P])

                        s1 = layer_part1(1, xTb[:], 0, w41[:], ag_in[0],
                                         ag_out[0], gloc_d[0], sp1, psp1, psg1,
                                         pss1, gp1, L1_TS,
                                         score_sink=score_sink)
                    with tc.tile_pool(name="ps_tr", bufs=2, space="PSUM") as ps_tr:
                        cd_full = {
                            "predib": lambda j: cst["predib"][:],
                            "sucib": lambda j: cst["sucib"][:],
                            "pred3": cst8["pred"][:],
                            "suc3": cst8["suc"][:],
                            "diagm3": cst8["diagm"][:],
                        }
                        a_build(ab, ps_tr, FULL_TS, B, NBL, cd_full,
                                sbF[:], smTF[:], cB[:], dB[:], "F", ATbF)
                        cd_mini = {
                            "predib": lambda j: cstm["predib"][:, j, :],
                            "sucib": lambda j: cstm["sucib"][:, j, :],
                            "pred3": cstm["pred"][:],
                            "suc3": cstm["suc"][:],
                            "diagm3": cstm["diagm"][:],
                        }
                        a_build(ab, ps_tr, MINI_TS, WIN, 2, cd_mini,
                                sbM[:], smTM[:], cM[:], dM[:], "M", ATbM)
                    layer_part2(1, xTb[:], 0, ag_out[0], gloc_d[0], s1, L1_TS)
            with tc.tile_pool(name="sL2", bufs=1) as sp2, \
                 tc.tile_pool(name="gL2", bufs=2) as gp2:
                with tc.tile_pool(name="psL2", bufs=3, space="PSUM") as psp2, \
                     tc.tile_pool(name="psG2", bufs=2, space="PSUM") as psg2, \
                     tc.tile_pool(name="psS2", bufs=1, space="PSUM") as pss2_p:
                    s2 = layer_part1(2, h1T[:], B, w42[:], ag_in[1], ag_out[1],
                                     gloc_d[1], sp2, psp2, psg2, pss2_p, gp2,
                                     FULL_TS)
                layer_part2(2, h1T[:], B, ag_out[1], gloc_d[1], s2, FULL_TS)
            head()

    split_multi_waits(nc)
    return nc


def split_multi_waits(nc, max_waits=1):
    """walrus only supports one sync-wait per instruction; hoist extras onto
    single-wait NoOps on the same engine queue."""
    n_fixed = 0
    for f in nc.m.functions:
        for bb in f.blocks:
            insts = list(bb.instructions)
            new_insts = []
            changed = False
            for ins in insts:
                si = getattr(ins, "sync_info", None)
                if si is not None and len(si.on_wait) > max_waits:
                    extra = list(si.on_wait)[:-max_waits]
                    keep = list(si.on_wait)[-max_waits:]
                    for j, w in enumerate(extra):
                        nop = mybir.InstNoOp(
                            name=f"wh{j}-{ins.name}", ins=[], outs=[],
                            engine=ins.engine,
                            sync_info=mybir.SyncInfo(on_wait=[w], on_update=[]),
                        )
                        new_insts.append(nop)
                    ins.sync_info = mybir.SyncInfo(
                        on_wait=keep, on_update=list(si.on_update))
                    changed = True
                    n_fixed += 1
                new_insts.append(ins)
            if changed:
                bb.instructions = new_insts
    return n_fixed


# ---------------- host-side input prep ----------------

def _consts_np():
    ii = np.arange(B)[:, None]
    cc = np.arange(EXT)[None, :]
    c = {}
    c["band"] = ((cc - ii >= 0) & (cc - ii <= 2 * WIN)).astype(np.float32)
    c["pred"] = ((cc - ii >= WIN) & (cc - ii <= 2 * WIN)).astype(np.float32)
    c["suc"] = ((cc - ii >= 0) & (cc - ii <= WIN - 1)).astype(np.float32)
    c["predib"] = ((cc >= ii + WIN) & (cc >= WIN) & (cc < WIN + B)).astype(np.float32)
    c["sucib"] = ((cc < ii + WIN) & (cc >= WIN) & (cc < WIN + B)).astype(np.float32)
    c["diagm"] = (cc == ii + WIN).astype(np.float32)
    cm = {}
    for n, v in c.items():
        cm[n] = np.stack([v[B - WIN:B], v[0:WIN]], axis=1).copy()  # [WIN, 2, EXT]
    return c, cm


def make_in_maps(inputs):
    x = np.asarray(inputs["x"], np.float32)
    spk = np.asarray(inputs["speakers"])
    E = np.zeros((N, NSPK), np.float32)
    E[np.arange(N), spk] = 1.0
    xg = np.zeros((N + 2 * HALO, D), np.float32)
    xg[HALO:HALO + N] = x
    Eg = np.zeros((N + 2 * HALO, NSPK), np.float32)
    Eg[HALO:HALO + N] = E

    bf = ml_dtypes.bfloat16
    w41 = np.concatenate([inputs["W_pred1"], inputs["W_suc1"],
                          inputs["W_same1"], inputs["W_diff1"]], axis=1)
    w42 = np.concatenate([inputs["W_pred2"], inputs["W_suc2"],
                          inputs["W_same2"], inputs["W_diff2"]], axis=1)
    shared = {
        "w41": np.asarray(w41, bf), "w42": np.asarray(w42, bf),
        "wag1": np.asarray(inputs["w_aggr_1"], bf),
        "wag2": np.asarray(inputs["w_aggr_2"], bf),
        "we1": np.asarray(inputs["w_e1"], bf),
        "we2": np.asarray(inputs["w_e2"], bf),
        "ws": np.asarray(inputs["w_s"], bf),
        "be1": np.asarray(inputs["b_e1"], np.float32).reshape(D, 1),
        "be2": np.asarray(inputs["b_e2"], np.float32).reshape(NEMO, 1),
        "bs": np.asarray(inputs["b_s"], np.float32).reshape(NEMO, 1),
    }
    cfull, cmini = _consts_np()
    for n in ("band", "predib", "sucib"):
        shared["c_" + n] = cfull[n]
    for n in ("pred", "suc", "diagm"):
        shared["c8_" + n] = np.tile(
            cfull[n][:, None, :], (1, NBL, 1)).reshape(B, NBL * EXT).copy()
    for n, v in cmini.items():
        shared["cm_" + n] = v

    in_maps = []
    for r in range(CORES):
        lo = r * R
        xc = xg[lo:lo + XR]
        Ec = Eg[lo:lo + XR]
        eTc = np.asarray(Ec.T, bf)
        eOz = np.zeros((NBL, EXT, NSPK1), np.float32)
        for t in range(NBL):
            es = B + B * t
            eOz[t, :, :NSPK] = Ec[es:es + EXT]
            eOz[t, :, NSPK] = Ec[es:es + EXT].sum(axis=1)
            eOz[t, :WIN] = 0.0
            eOz[t, WIN + B:] = 0.0
        eOc = np.asarray(eOz.reshape(NBL * EXT, NSPK1), bf)
        # e4 rows: [+E, +E, -E, -E, 1, 1] -> cross = H0[c]+H1[c]
        #   + (totS - H2[c]) + (totP - H3[c]) in a single contraction
        Ecen = Ec[HALO:HALO + R].T
        e4T = np.concatenate(
            [Ecen, Ecen, -Ecen, -Ecen, np.ones((2, R), np.float32)], axis=0)
        Em = np.concatenate(
            [Ec[B:B + WIN], Ec[HALO + R:HALO + R + WIN]], axis=0).T
        e4Tm = np.concatenate(
            [Em, Em, -Em, -Em, np.ones((2, 2 * WIN), np.float32)], axis=0)
        # local (per-block) triangular weights: cols 0-7 full blocks,
        # col 8 = left mini (gblk r*8-1), col 9 = right mini (gblk (r+1)*8)
        J8 = np.arange(NBL)[:, None]
        T8 = np.arange(NBL)[None, :]
        tls = np.zeros((NBL, 10), np.float32)
        tls[:, :NBL] = (J8 > T8)
        tls[:, 8] = 1.0
        tlp = np.zeros((NBL, 10), np.float32)
        tlp[:, :NBL] = (J8 < T8)
        tlp[:, 9] = 1.0
        triL = np.stack([tls, tlp], axis=1)
        # remote weights over gathered slots [sum, first, last] per core,
        # with edge-block corrections for the mini columns
        trs = np.zeros((3 * CORES, 10), np.float32)
        trp = np.zeros((3 * CORES, 10), np.float32)
        for rr in range(CORES):
            if rr > r:
                trs[3 * rr, :] = 1.0
            if rr < r:
                trp[3 * rr, :] = 1.0
        if r + 1 < CORES:
            trs[3 * (r + 1) + 1, 9] = -1.0
        if r - 1 >= 0:
            trp[3 * (r - 1) + 2, 8] = -1.0
        triR = np.stack([trs, trp], axis=1)
        vm = np.ones((WIN, 2), np.float32)
        if r == 0:
            vm[:, 0] = 0.0
        if r == CORES - 1:
            vm[:, 1] = 0.0
        m = dict(shared)
        m.update({
            "xT": np.ascontiguousarray(xc.T),
            "eT": eTc, "eO": eOc,
            "e4T": np.asarray(e4T, bf), "e4Tm": np.asarray(e4Tm, bf),
            "triL": np.asarray(triL, bf),
            "triR": np.asarray(triR, bf),
            "vmask": vm,
        })
        in_maps.append(m)
    return in_maps


_NC = None


def kernel(**inputs):
    global _NC
    if _NC is None:
        _NC = build_program()
    in_maps = make_in_maps(inputs)
    res = run_bass_kernel_spmd(_NC, in_maps, list(range(CORES)))
    emo = np.concatenate(
        [np.asarray(res.results[r]["emo"]).T for r in range(CORES)], axis=0)
    sen = np.concatenate(
        [np.asarray(res.results[r]["sen"]).T for r in range(CORES)], axis=0)
    return emo, sen



# revision 41
# speedup vs baseline: 1.1240x; 1.1240x over previous
"""DialogueGCN Trainium2 kernel — 8-core SPMD row-sharded implementation.

Decomposition (validated in numpy):
  attn = softmax(band(x@x.T)) has off-band entries equal to a per-row constant
  c_i = exp(-m_i)/Z_i.  Each relation adjacency adj_k = mask_k * attn splits into
    adj_k @ s = [A_k^ext @ s_ext]   (per-96-row-block: c_i*mask within own block
                                     + band corrections over +-10 cols)
    + c_i * (E_rows @ H_k)          (cross-block per-speaker-class prefix/suffix
                                     sums of s, via a tiny AllGather of per-block
                                     class sums G)
  Mini-blocks (10 halo rows each side) replicate neighbour-core h1 rows locally
  so layer 2 needs no halo exchange.
"""
import os
import sys

for _p in ("/opt/trn_rl_repo", "/root/.axon_site/_ro/trn_rl_repo"):
    if os.path.isdir(_p) and _p not in sys.path:
        sys.path.insert(0, _p)

import numpy as np
import ml_dtypes

import concourse.bass as bass
import concourse.mybir as mybir
import concourse.tile as tile
from concourse import masks
from concourse.bass_utils import run_bass_kernel_spmd

N, D, WIN, NSPK, NEMO = 6144, 128, 10, 8, 7
NSPK1 = NSPK + 1           # 8 speaker classes + all-ones "block total" class
E4R = 4 * NSPK + 2         # e4 rows: 4 rels x 8 classes + totS + totP
CORES, R, B, NBL = 8, 768, 96, 8
EXT = B + 2 * WIN          # 116
HALO = B + WIN             # 106
XR = R + 2 * HALO          # 980
NBG = CORES * NBL          # 64
F32 = mybir.dt.float32
BF16 = mybir.dt.bfloat16
AOT = mybir.AluOpType
ACTF = mybir.ActivationFunctionType

# block geometry: (t, ostart, P, estart, mini_col)  in local l coords
FULL_TS = [(t, HALO + B * t, B, B + B * t, None) for t in range(NBL)]
MINI_TS = [(8, B, WIN, 0, 0), (9, HALO + R, WIN, HALO + R - WIN - B, 1)]
# mini R: rows l in [874, 884), ext cols [864, 980) -> estart = 864 = HALO+R-WIN-B? 106+768-10-96=768? no:
MINI_TS = [(8, B, WIN, 0, 0), (9, HALO + R, WIN, XR - EXT, 1)]


def _bcast(ap, shape):
    return ap.broadcast_to(shape)


def build_program():
    nc = bass.Bass()
    dp = nc.declare_dram_parameter

    xT_d = dp("xT", [D, XR], F32, isOutput=False)
    eT_d = dp("eT", [NSPK, XR], BF16, isOutput=False)
    eO_d = dp("eO", [NBL * EXT, NSPK1], BF16, isOutput=False)
    e4T_d = dp("e4T", [E4R, R], BF16, isOutput=False)
    e4Tm_d = dp("e4Tm", [E4R, 2 * WIN], BF16, isOutput=False)
    w41_d = dp("w41", [D, 4 * D], BF16, isOutput=False)
    w42_d = dp("w42", [D, 4 * D], BF16, isOutput=False)
    wag1_d = dp("wag1", [D, D], BF16, isOutput=False)
    wag2_d = dp("wag2", [D, D], BF16, isOutput=False)
    we1_d = dp("we1", [2 * D, D], BF16, isOutput=False)
    we2_d = dp("we2", [D, NEMO], BF16, isOutput=False)
    ws_d = dp("ws", [2 * D, NEMO], BF16, isOutput=False)
    be1_d = dp("be1", [D, 1], F32, isOutput=False)
    be2_d = dp("be2", [NEMO, 1], F32, isOutput=False)
    bs_d = dp("bs", [NEMO, 1], F32, isOutput=False)
    # shape constants: single-block [B, EXT], block-tiled [B, NBL*EXT],
    # mini variants [WIN, 2, EXT]
    cnames = ["band", "pred", "suc", "predib", "sucib", "diagm"]
    c_d = {n: dp("c_" + n, [B, EXT], F32, isOutput=False)
           for n in ("band", "predib", "sucib")}
    c8_d = {n: dp("c8_" + n, [B, NBL * EXT], F32, isOutput=False)
            for n in ("pred", "suc", "diagm")}
    cm_d = {n: dp("cm_" + n, [WIN, 2, EXT], F32, isOutput=False) for n in cnames}
    triL_d = dp("triL", [NBL, 2, 10], BF16, isOutput=False)
    triR_d = dp("triR", [3 * CORES, 2, 10], BF16, isOutput=False)
    vmask_d = dp("vmask", [WIN, 2], F32, isOutput=False)
    emo_d = dp("emo", [NEMO, R], F32, isOutput=True)
    sen_d = dp("sen", [NEMO, R], F32, isOutput=True)

    # AllGather payload: 3 slots per core = [core-sum, first block G,
    # last block G]; per-block G stays in a LOCAL dram buffer.
    ag_in = [nc.dram_tensor(f"ag{L}_in", [3, NSPK1, 4 * D], BF16) for L in (1, 2)]
    ag_out = [
        nc.dram_tensor(f"ag{L}_out", [3 * CORES, NSPK1, 4 * D], BF16,
                       addr_space="Shared")
        for L in (1, 2)
    ]
    gloc_d = [nc.dram_tensor(f"gloc{L}", [NBL, NSPK1, 4 * D], BF16)
              for L in (1, 2)]

    with tile.TileContext(nc) as tc:
        with tc.tile_pool(name="persist", bufs=1) as pp, \
             tc.tile_pool(name="cpool", bufs=1) as cp:
            # ---- load inputs / constants ----
            # distribute input-load DMA triggers across engine queues: the
            # SP queue pays ~620ns per trigger, so serializing ~30 loads
            # there delays everything queued behind them.
            _eng_rr = [nc.sync, nc.gpsimd, nc.scalar, nc.sync, nc.gpsimd]
            _eng_i = [0]

            def _load(out, in_):
                _eng_rr[_eng_i[0] % len(_eng_rr)].dma_start(out=out, in_=in_)
                _eng_i[0] += 1

            xT = pp.tile([D, XR], F32)
            nc.sync.dma_start(out=xT[:, 0:490], in_=xT_d[:, 0:490])
            nc.scalar.dma_start(out=xT[:, 490:XR], in_=xT_d[:, 490:XR])
            xTb = pp.tile([D, XR], BF16)
            nc.vector.tensor_copy(xTb[:], xT[:])
            eT = pp.tile([NSPK, XR], BF16)
            _load(eT[:], eT_d[:])
            eO = pp.tile([EXT, NBL, NSPK1], BF16)
            _load(eO[:], eO_d[:].rearrange("(b p) c -> p b c", p=EXT))
            e4T = pp.tile([E4R, R], BF16)
            _load(e4T[:], e4T_d[:])
            e4Tm = pp.tile([E4R, 2 * WIN], BF16)
            _load(e4Tm[:], e4Tm_d[:])
            w41 = pp.tile([D, 4 * D], BF16)
            nc.gpsimd.dma_start(out=w41[:], in_=w41_d[:])
            w42 = pp.tile([D, 4 * D], BF16)
            _load(w42[:], w42_d[:])
            wag1 = pp.tile([D, D], BF16)
            _load(wag1[:], wag1_d[:])
            wag2 = pp.tile([D, D], BF16)
            _load(wag2[:], wag2_d[:])
            we1a = pp.tile([D, D], BF16)
            _load(we1a[:], we1_d[0:D, :])
            we1b = pp.tile([D, D], BF16)
            _load(we1b[:], we1_d[D:2 * D, :])
            we2 = pp.tile([D, NEMO], BF16)
            _load(we2[:], we2_d[:])
            wsa = pp.tile([D, NEMO], BF16)
            _load(wsa[:], ws_d[0:D, :])
            wsb = pp.tile([D, NEMO], BF16)
            _load(wsb[:], ws_d[D:2 * D, :])
            be1 = pp.tile([D, 1], F32)
            _load(be1[:], be1_d[:])
            be2 = pp.tile([NEMO, 1], F32)
            _load(be2[:], be2_d[:])
            bs = pp.tile([NEMO, 1], F32)
            _load(bs[:], bs_d[:])
            cst = {}
            for n in ("band", "predib", "sucib"):
                cst[n] = cp.tile([B, EXT], F32, name="c_" + n)
                _load(cst[n][:], c_d[n][:])
            cst8 = {}
            for n in ("pred", "suc", "diagm"):
                cst8[n] = cp.tile([B, NBL, EXT], F32, name="c8_" + n)
                _load(cst8[n][:],
                      c8_d[n][:].rearrange("p (b e) -> p b e", e=EXT))
            cstm = {}
            for n in cnames:
                cstm[n] = cp.tile([WIN, 2, EXT], F32, name="cm_" + n)
                _load(cstm[n][:], cm_d[n][:])
            triLS = pp.tile([NBL, 10], BF16)
            _load(triLS[:], triL_d[:, 0, :])
            triLP = pp.tile([NBL, 10], BF16)
            _load(triLP[:], triL_d[:, 1, :])
            triRS = pp.tile([3 * CORES, 10], BF16)
            _load(triRS[:], triR_d[:, 0, :])
            triRP = pp.tile([3 * CORES, 10], BF16)
            _load(triRP[:], triR_d[:, 1, :])
            vmask = pp.tile([WIN, 2], F32)
            _load(vmask[:], vmask_d[:])
            idf = pp.tile([128, 128], F32)
            masks.make_identity(nc, idf[:])
            idb = pp.tile([128, 128], BF16)
            masks.make_identity(nc, idb[:])

            # ---- persistent state tiles ----
            h1T = pp.tile([D, R + 2 * WIN], BF16)       # col = l - 96
            h2T = pp.tile([D, R], BF16)
            cB = pp.tile([B, NBL], F32)
            dB = pp.tile([B, NBL], F32)
            cM = pp.tile([WIN, 2], F32)
            dM = pp.tile([WIN, 2], F32)
            # A^T batch tiles: [EXT, nb, P] per relation; AT[(k, t)] are APs
            ATbF = {k: pp.tile([EXT, NBL, B], BF16, name=f"ATF{k}")
                    for k in range(4)}
            ATbM = {k: pp.tile([EXT, 2, WIN], BF16, name=f"ATM{k}")
                    for k in range(4)}
            AT = {}
            for t, _, P, _, _ in FULL_TS:
                for k in range(4):
                    AT[(k, t)] = ATbF[k][:, t, :]
            for t, _, P, _, _ in MINI_TS:
                for k in range(4):
                    AT[(k, t)] = ATbM[k][:, t - 8, :]
            accM = {}
            accA = {}
            for t, _, P, _, _ in FULL_TS + MINI_TS:
                accM[(t, 1)] = pp.tile([P, D], F32, name=f"accM1_{t}")
                accA[(t, 1)] = pp.tile([P, D], F32, name=f"accA1_{t}")
                if t < NBL:
                    accM[(t, 2)] = pp.tile([P, D], F32, name=f"accM2_{t}")
                    accA[(t, 2)] = pp.tile([P, D], F32, name=f"accA2_{t}")

            # ---------- helpers ----------
            SPL = 6      # elementwise split: blocks [0:SPL] on DVE, rest GpSimd

            def split_tt(out, in0, in1, op, nb):
                """emit a batched [P, nb, EXT] tensor_tensor split DVE/GpSimd"""
                if nb <= 2 or SPL >= nb:
                    nc.vector.tensor_tensor(out, in0, in1, op)
                    return
                nc.vector.tensor_tensor(
                    out[:, 0:SPL, :], in0[:, 0:SPL, :], in1[:, 0:SPL, :], op)
                nc.gpsimd.tensor_tensor(
                    out[:, SPL:nb, :], in0[:, SPL:nb, :], in1[:, SPL:nb, :], op)

            # =============== layer part 1: s, G, AllGather (+ scores L1) =======
            # The G-chain (per-block class sums -> AllGather input) is emitted
            # FIRST under high_priority so the collective triggers as early as
            # its data allows and overlaps with the attention math, instead of
            # queueing behind it.
            def layer_part1(L, hT, hoff, w4, agi, ago, gloc, sp, psp, psg, pss_p,
                            gp, ts_list, score_sink=None):
                s_tiles = {}
                dmae = nc.gpsimd if L == 1 else nc.sync
                with tc.high_priority():
                    if L == 1:
                        psSum = pss_p.tile([NSPK1, 4 * D], F32, name=f"psS{L}")
                    else:
                        # L2 runs inside the PE-saturated window: accumulate
                        # the core-sum on the DVE instead (it has slack there)
                        accS = gp.tile([NSPK1, 4 * D], F32, name="accS2")
                    for t, ostart, P, estart, _ in ts_list:
                        if t >= NBL:
                            continue
                        pss = psp.tile([EXT, 4 * D], F32, name=f"pss{L}",
                                       tag="pss")
                        nc.tensor.matmul(
                            pss[:], hT[:, estart - hoff:estart - hoff + EXT],
                            w4[:], start=True, stop=True)
                        sAll = sp.tile([EXT, 4 * D], BF16, name=f"sAll{L}_{t}")
                        (nc.scalar.copy if t % 2 else nc.vector.tensor_copy)(
                            sAll[:], pss[:])
                        s_tiles[t] = sAll
                        ps2 = psg.tile([NSPK1, 4 * D], F32, name=f"psg{L}",
                                       tag="psg")
                        nc.tensor.matmul(
                            ps2[:], eO[:, t, :], sAll[:], start=True, stop=True)
                        if L == 1:
                            # accumulate core-sum (one PSUM group over blocks)
                            nc.tensor.matmul(
                                psSum[:], eO[:, t, :], sAll[:],
                                start=(t == 0), stop=(t == NBL - 1))
                        gsb = gp.tile([NSPK1, 4 * D], BF16, name=f"gsb{L}",
                                      tag="gsb")
                        (nc.vector.tensor_copy if t % 2 else nc.scalar.copy)(
                            gsb[:], ps2[:])
                        if L == 2:
                            if t == 0:
                                nc.vector.tensor_copy(accS[:], ps2[:])
                            else:
                                nc.vector.tensor_tensor(
                                    accS[:], accS[:], ps2[:], AOT.add)
                        dmae.dma_start(out=gloc[t], in_=gsb[:])
                        if t == 0:
                            dmae.dma_start(out=agi[1], in_=gsb[:])
                        elif t == NBL - 1:
                            dmae.dma_start(out=agi[2], in_=gsb[:])
                    gsum = gp.tile([NSPK1, 4 * D], BF16, name=f"gsum{L}",
                                   tag="gsum")
                    if L == 1:
                        nc.scalar.copy(gsum[:], psSum[:])
                    else:
                        nc.scalar.copy(gsum[:], accS[:])
                    dmae.dma_start(out=agi[0], in_=gsum[:])
                    nc.gpsimd.collective_compute(
                        "AllGather", AOT.bypass,
                        replica_groups=[list(range(CORES))],
                        ins=[agi[:]], outs=[ago[:]],
                    )
                for i, (t, ostart, P, estart, _) in enumerate(ts_list):
                    if t >= NBL:
                        pss = psp.tile([EXT, 4 * D], F32, name=f"pss{L}",
                                       tag="pss")
                        nc.tensor.matmul(
                            pss[:], hT[:, estart - hoff:estart - hoff + EXT],
                            w4[:], start=True, stop=True)
                        sAll = sp.tile([EXT, 4 * D], BF16, name=f"sAll{L}_{t}")
                        if i % 2 == 0:
                            nc.vector.tensor_copy(sAll[:], pss[:])
                        else:
                            nc.scalar.copy(sAll[:], pss[:])
                        s_tiles[t] = sAll
                    pag = psg.tile([B, D], F32, name=f"pag{L}", tag="pag")
                    nc.tensor.matmul(
                        pag[:P, :], hT[:, ostart - hoff:ostart - hoff + P],
                        (wag1 if L == 1 else wag2)[:], start=True, stop=True)
                    nc.vector.tensor_copy(accA[(t, L)][:], pag[:P, :])
                    if score_sink is not None:
                        score_sink(t, ostart, P, estart)
                return s_tiles

            # =============== attention math (layer independent) ===============
            def a_build(ab, ps_tr, blocks, PP, nb, cd, sb, smT, c_out, d_out,
                        tag, ATt):
                """sb: [PP, nb, EXT] banded scores; smT: [EXT, nb, PP]
                transposed same-speaker masks (bf16). cd: 'predib','sucib' ->
                per-block [P,EXT] AP fns; 'pred3','suc3','diagm3' ->
                [PP, nb, EXT] real-tile APs. ATt: k -> [EXT, nb, PP] output."""
                sh3 = [PP, nb, EXT]
                mB = ab.tile([PP, nb], F32, name=f"mB{tag}")       # holds -m
                nc.vector.tensor_reduce(
                    mB[:], sb[:], axis=mybir.AxisListType.X, op=AOT.max,
                    negate=True)
                exv = ab.tile(sh3, F32, name=f"exv{tag}")
                sumB = ab.tile([PP, nb], F32, name=f"sumB{tag}")
                for j in range(nb):
                    # exp(s - m) in one ACT op: out = Exp(in + bias), bias = -m
                    nc.scalar.activation(
                        exv[:, j, :], sb[:, j, :], ACTF.Exp,
                        bias=mB[:, j:j + 1], accum_out=sumB[:, j:j + 1])
                enB = ab.tile([PP, nb], F32, name=f"enB{tag}")
                nc.scalar.activation(enB[:], mB[:], ACTF.Exp)
                ZB = ab.tile([PP, nb], F32, name=f"ZB{tag}")
                nc.vector.scalar_tensor_tensor(
                    ZB[:], enB[:], float(N - EXT), sumB[:], AOT.mult, AOT.add)
                rZ = ab.tile([PP, nb], F32, name=f"rZ{tag}")
                nc.vector.reciprocal(rZ[:], ZB[:])
                nc.vector.tensor_tensor(c_out, enB[:], rZ[:], AOT.mult)
                dg = ab.tile(sh3, F32, name=f"dg{tag}")
                split_tt(dg[:], exv[:], cd["diagm3"], AOT.mult, nb)
                d0 = ab.tile([PP, nb], F32, name=f"d0{tag}")
                nc.vector.tensor_reduce(
                    d0[:], dg[:], axis=mybir.AxisListType.X, op=AOT.add)
                nc.vector.tensor_tensor(d_out, d0[:], rZ[:], AOT.mult)
                u = ab.tile(sh3, F32, name=f"u{tag}")
                for j in range(nb):
                    nc.vector.tensor_scalar(
                        u[:, j, :], exv[:, j, :], enB[:, j:j + 1], rZ[:, j:j + 1],
                        AOT.subtract, AOT.mult)
                up = ab.tile(sh3, F32, name=f"up{tag}")
                split_tt(up[:], u[:], cd["pred3"], AOT.mult, nb)
                un = ab.tile(sh3, F32, name=f"un{tag}")
                split_tt(un[:], u[:], cd["suc3"], AOT.mult, nb)
                # w1/w2 in bf16: sm is exactly 0/1, so rounding w before the
                # mask multiply gives bit-identical A_k to rounding after.
                w1 = ab.tile(sh3, BF16, name=f"w1{tag}")
                w2 = ab.tile(sh3, BF16, name=f"w2{tag}")
                for j in range(nb):
                    nc.vector.scalar_tensor_tensor(
                        w1[:, j, :], cd["predib"](j), c_out[:, j:j + 1],
                        up[:, j, :], AOT.mult, AOT.add)
                    nc.vector.scalar_tensor_tensor(
                        w2[:, j, :], cd["sucib"](j), c_out[:, j:j + 1],
                        un[:, j, :], AOT.mult, AOT.add)
                # transpose w1/w2 per block, then do the same/diff mask algebra
                # in transposed bf16 layout (A3 = w1 - A1, A4 = w2 - A2)
                w1T = ab.tile([EXT, nb, PP], BF16, name=f"w1T{tag}")
                w2T = ab.tile([EXT, nb, PP], BF16, name=f"w2T{tag}")
                for j, (t, ostart, P, estart, _) in enumerate(blocks):
                    for src, dstT in ((w1, w1T), (w2, w2T)):
                        pst = ps_tr.tile([EXT, PP], BF16, name="pst", tag="pst")
                        nc.tensor.matmul(
                            pst[:, :P], src[:P, j, :], idb[:P, :P],
                            is_transpose=True, start=True, stop=True)
                        nc.any.tensor_copy(dstT[:, j, :P], pst[:, :P])
                nc.vector.tensor_tensor(ATt[0][:], smT[:], w1T[:], AOT.mult)
                nc.vector.tensor_tensor(ATt[1][:], smT[:], w2T[:], AOT.mult)
                nc.vector.tensor_tensor(ATt[2][:], w1T[:], ATt[0][:],
                                        AOT.subtract)
                nc.vector.tensor_tensor(ATt[3][:], w2T[:], ATt[1][:],
                                        AOT.subtract)

            def part2_order(ts_list):
                if len(ts_list) <= NBL:
                    return ts_list
                by_t = {t[0]: t for t in ts_list}
                order = [8, 0, 1, 2, 3, 4, 5, 6, 9, 7]
                return [by_t[t] for t in order]

            # =============== layer part 2: A-matmuls, H, cross, combine ========
            def layer_part2(L, hT, hoff, ago, gloc, s_tiles, ts_list):
                ts_list = part2_order(ts_list)
                with tc.tile_pool(name=f"psA{L}", bufs=3, space="PSUM") as psa:
                    for t, ostart, P, estart, mcol in ts_list:
                        pm = psa.tile([P, D], F32, name=f"pm{L}", tag="pm")
                        for k in range(4):
                            nc.tensor.matmul(
                                pm[:], AT[(k, t)],
                                s_tiles[t][:, k * D:(k + 1) * D],
                                start=(k == 0), stop=(k == 3))
                        dsl = (dB[:, t:t + 1] if t < NBL
                               else dM[:, mcol:mcol + 1])
                        # accC = aggr*d + sum_k A_k @ s_k
                        nc.vector.scalar_tensor_tensor(
                            accM[(t, L)][:], accA[(t, L)][:], dsl, pm[:],
                            AOT.mult, AOT.add)
                with tc.tile_pool(name=f"hL{L}", bufs=1) as hp:
                    gf = hp.tile([3 * CORES, NSPK1, 4, D], BF16, name=f"gf{L}")
                    ago_v = ago[:].rearrange("g c (r d) -> g c r d", r=4)
                    # local per-block G (no collective dependency): the local
                    # half of every H prefix-sum can run during the AllGather
                    gl = hp.tile([NBL, NSPK1, 4, D], BF16, name=f"gl{L}")
                    nc.scalar.dma_start(
                        out=gl[:],
                        in_=gloc[:].rearrange("g c (r d) -> g c r d", r=4))
                    # fence: gf loads (and so every gf-dependent matmul) only
                    # become schedulable after the last A-matmul combine, so
                    # the PE queue keeps all overlap work AHEAD of the
                    # collective-gated instructions (avoids head-of-line
                    # blocking during the AllGather).
                    fence = hp.tile([1, 1], F32, name=f"fence{L}")
                    nc.gpsimd.tensor_copy(
                        fence[:], accM[(ts_list[-1][0], L)][0:1, 0:1])
                    nc.gpsimd.dma_start(out=gf[:], in_=ago_v[:])
                    # hcat slots: [rel*8+c] raw per-class H, 32/33 = block
                    # totals (tot - H_c is folded into the pc contraction via
                    # negated e4 rows + ones rows for the tot slots).
                    hcat = hp.tile([10, E4R, D], BF16, name=f"hcat{L}")
                    h_srcs = [
                        (0, triLS, triRS, 0),   # k=1 same-pred
                        (1, triLP, triRP, 1),   # k=2 same-suc
                        (2, triLS, triRS, 2),   # k=3 diff-pred
                        (3, triLP, triRP, 3),   # k=4 diff-suc
                    ]
                    with tc.tile_pool(name=f"psH{L}", bufs=6, space="PSUM") as psh:
                        for rel, tl, tr, rr in h_srcs:
                            for c0 in (0, 4):
                                ph = psh.tile([10, 4 * D], F32, name=f"ph{L}",
                                              tag="ph")
                                nc.tensor.matmul(
                                    ph[:], tl[:], gl[:, c0:c0 + 4, rr, :],
                                    start=True, stop=False)
                                nc.tensor.matmul(
                                    ph[:], tr[:], gf[:, c0:c0 + 4, rr, :],
                                    start=False, stop=True)
                                s0 = rel * NSPK + c0
                                (nc.vector.tensor_copy if c0 else nc.scalar.copy)(
                                    hcat[:, s0:s0 + 4, :],
                                    ph[:].rearrange("p (c d) -> p c d", d=D))
                        pt = psh.tile([10, 2 * D], F32, name=f"pt{L}", tag="ph")
                        nc.tensor.matmul(pt[:, 0:D], triLS[:],
                                         gl[:, NSPK, 2, :],
                                         start=True, stop=False)
                        nc.tensor.matmul(pt[:, 0:D], triRS[:],
                                         gf[:, NSPK, 2, :],
                                         start=False, stop=True)
                        nc.tensor.matmul(pt[:, D:2 * D], triLP[:],
                                         gl[:, NSPK, 3, :],
                                         start=True, stop=False)
                        nc.tensor.matmul(pt[:, D:2 * D], triRP[:],
                                         gf[:, NSPK, 3, :],
                                         start=False, stop=True)
                        nc.scalar.copy(
                            hcat[:, 4 * NSPK:E4R, :],
                            pt[:].rearrange("p (c d) -> p c d", d=D))
                    with tc.tile_pool(name=f"xb{L}", bufs=1) as xb, \
                         tc.tile_pool(name=f"psX{L}", bufs=2, space="PSUM") as psx:
                        hm4s = {}
                        _dq = [nc.sync, nc.gpsimd, nc.scalar]
                        for di, (t, ostart, P, estart, mcol) in enumerate(ts_list):
                            hm4 = xb.tile([E4R, D], BF16, name=f"hm4{L}_{t}")
                            _dq[di % 3].dma_start(
                                out=hm4[:], in_=hcat[t:t + 1, :, :])
                            hm4s[t] = hm4
                        for t, ostart, P, estart, mcol in ts_list:
                            pc = psx.tile([P, D], F32, name=f"pc{L}", tag="pc",
                                          bufs=4)
                            if t < NBL:
                                e4sl = e4T[:, B * t:B * t + P]
                            else:
                                e4sl = e4Tm[:, mcol * WIN:(mcol + 1) * WIN]
                            nc.tensor.matmul(
                                pc[:], e4sl, hm4s[t][:],
                                start=True, stop=True)
                            csl = (cB[:, t:t + 1] if t < NBL
                                   else cM[:, mcol:mcol + 1])
                            hrow = xb.tile([P, D], F32, name=f"hrow{L}",
                                           tag="hrow", bufs=6)
                            nc.vector.scalar_tensor_tensor(
                                hrow[:], pc[:], csl, accM[(t, L)][:],
                                AOT.mult, AOT.add)
                            if t >= NBL:
                                nc.vector.tensor_scalar_mul(
                                    hrow[:], hrow[:], vmask[:, mcol:mcol + 1])
                            ptr = psx.tile([D, P], F32, name=f"ptr{L}", tag="ptr",
                                           bufs=4)
                            nc.tensor.matmul(
                                ptr[:], hrow[:], idf[:P, :P],
                                is_transpose=True, start=True, stop=True)
                            if L == 1:
                                off = {8: 0, 9: R + WIN}.get(t, WIN + B * t)
                                nc.scalar.activation(
                                    h1T[:, off:off + P], ptr[:], ACTF.Relu)
                            else:
                                nc.scalar.activation(
                                    h2T[:, B * t:B * t + P], ptr[:], ACTF.Relu)

            # =============== head: two 384-wide chunks over h2T ===============
            def head():
                CH = 4 * B
                with tc.tile_pool(name="hd", bufs=3) as hd, \
                     tc.tile_pool(name="psE", bufs=3, space="PSUM") as pse:
                    for c0 in (0, CH):
                        h2c = h2T[:, c0:c0 + CH]
                        xc_ = xTb[:, HALO + c0:HALO + c0 + CH]
                        pe1 = pse.tile([D, CH], F32, name="pe1", tag="pe1")
                        nc.tensor.matmul(pe1[:], we1a[:], h2c,
                                         start=True, stop=False)
                        nc.tensor.matmul(pe1[:], we1b[:], xc_,
                                         start=False, stop=True)
                        e1b = hd.tile([D, CH], BF16, name="e1b", tag="e1b")
                        nc.scalar.activation(e1b[:], pe1[:], ACTF.Relu,
                                             bias=be1[:])
                        pe2 = pse.tile([NEMO, CH], F32, name="pe2", tag="pe2")
                        nc.tensor.matmul(pe2[:], we2[:], e1b[:],
                                         start=True, stop=True)
                        em1 = hd.tile([NEMO, CH], F32, name="em1", tag="em1")
                        nc.vector.tensor_scalar_add(em1[:], pe2[:], be2[:])
                        ps2 = pse.tile([NEMO, CH], F32, name="ps2", tag="pe2")
                        nc.tensor.matmul(ps2[:], wsa[:], h2c,
                                         start=True, stop=False)
                        nc.tensor.matmul(ps2[:], wsb[:], xc_,
                                         start=False, stop=True)
                        sn1 = hd.tile([NEMO, CH], F32, name="sn1", tag="em1")
                        nc.vector.tensor_scalar_add(sn1[:], ps2[:], bs[:])
                        # outputs stored transposed [NEMO, R]; host transposes
                        nc.sync.dma_start(out=emo_d[:, c0:c0 + CH], in_=em1[:])
                        nc.scalar.dma_start(out=sen_d[:, c0:c0 + CH], in_=sn1[:])

            # =============== orchestrate ===============
            L1_TS = FULL_TS + MINI_TS
            with tc.tile_pool(name="abuild", bufs=1) as ab:
                sbF = ab.tile([B, NBL, EXT], F32, name="sbF")
                smTF = ab.tile([EXT, NBL, B], BF16, name="smTF")
                sbM = ab.tile([WIN, 2, EXT], F32, name="sbM")
                smTM = ab.tile([EXT, 2, WIN], BF16, name="smTM")
                with tc.tile_pool(name="sL1", bufs=1) as sp1, \
                     tc.tile_pool(name="gL1", bufs=2) as gp1:
                    with tc.tile_pool(name="psL1", bufs=3, space="PSUM") as psp1, \
                         tc.tile_pool(name="psG1", bufs=1, space="PSUM") as psg1, \
                         tc.tile_pool(name="psS1", bufs=1, space="PSUM") as pss1, \
                         tc.tile_pool(name="ps_sc", bufs=1, space="PSUM") as ps_sc, \
                         tc.tile_pool(name="ps_sm", bufs=1, space="PSUM") as ps_sm:

                        def score_sink(t, ostart, P, estart):
                            j = t if t < NBL else t - NBL
                            sb_t = sbF if t < NBL else sbM
                            smT_t = smTF if t < NBL else smTM
                            bandap = (cst["band"][:] if t < NBL
                                      else cstm["band"][:, j, :])
                            pssc = ps_sc.tile([B, EXT], F32, name="pssc",
                                              tag="pssc")
                            nc.tensor.matmul(
                                pssc[:P, :], xT[:, ostart:ostart + P],
                                xT[:, estart:estart + EXT], start=True,
                                stop=True)
                            nc.vector.tensor_tensor(
                                sb_t[:P, j, :], pssc[:P, :], bandap[:P],
                                AOT.mult)
                            # speaker-same mask, produced directly transposed
                            pssm = ps_sm.tile([EXT, B], F32, name="pssm",
                                              tag="pssm")
                            nc.tensor.matmul(
                                pssm[:, :P], eT[:, estart:estart + EXT],
                                eT[:, ostart:ostart + P], start=True,
                                stop=True)
                            (nc.vector.tensor_copy if j % 2 else nc.scalar.copy)(
                                smT_t[:, j, :P], pssm[:, :P])

                        s1 = layer_part1(1, xTb[:], 0, w41[:], ag_in[0],
                                         ag_out[0], gloc_d[0], sp1, psp1, psg1,
                                         pss1, gp1, L1_TS,
                                         score_sink=score_sink)
                    with tc.tile_pool(name="ps_tr", bufs=2, space="PSUM") as ps_tr:
                        cd_full = {
                            "predib": lambda j: cst["predib"][:],
                            "sucib": lambda j: cst["sucib"][:],
                            "pred3": cst8["pred"][:],
                            "suc3": cst8["suc"][:],
                            "diagm3": cst8["diagm"][:],
                        }
                        a_build(ab, ps_tr, FULL_TS, B, NBL, cd_full,
                                sbF[:], smTF[:], cB[:], dB[:], "F", ATbF)
                        cd_mini = {
                            "predib": lambda j: cstm["predib"][:, j, :],
                            "sucib": lambda j: cstm["sucib"][:, j, :],
                            "pred3": cstm["pred"][:],
                            "suc3": cstm["suc"][:],
                            "diagm3": cstm["diagm"][:],
                        }
                        a_build(ab, ps_tr, MINI_TS, WIN, 2, cd_mini,
                                sbM[:], smTM[:], cM[:], dM[:], "M", ATbM)
                    layer_part2(1, xTb[:], 0, ag_out[0], gloc_d[0], s1, L1_TS)
            with tc.tile_pool(name="sL2", bufs=1) as sp2, \
                 tc.tile_pool(name="gL2", bufs=2) as gp2:
                with tc.tile_pool(name="psL2", bufs=3, space="PSUM") as psp2, \
                     tc.tile_pool(name="psG2", bufs=2, space="PSUM") as psg2, \
                     tc.tile_pool(name="psS2", bufs=1, space="PSUM") as pss2_p:
                    s2 = layer_part1(2, h1T[:], B, w42[:], ag_in[1], ag_out[1],
                                     gloc_d[1], sp2, psp2, psg2, pss2_p, gp2,
                                     FULL_TS)
                layer_part2(2, h1T[:], B, ag_out[1], gloc_d[1], s2, FULL_TS)
            head()

    split_multi_waits(nc)
    return nc


def split_multi_waits(nc, max_waits=1):
    """walrus only supports one sync-wait per instruction; hoist extras onto
    single-wait NoOps on the same engine queue."""
    n_fixed = 0
    for f in nc.m.functions:
        for bb in f.blocks:
            insts = list(bb.instructions)
            new_insts = []
            changed = False
            for ins in insts:
                si = getattr(ins, "sync_info", None)
                if si is not None and len(si.on_wait) > max_waits:
                    extra = list(si.on_wait)[:-max_waits]
                    keep = list(si.on_wait)[-max_waits:]
                    for j, w in enumerate(extra):
                        nop = mybir.InstNoOp(
                            name=f"wh{j}-{ins.name}", ins=[], outs=[],
                            engine=ins.engine,
                            sync_info=mybir.SyncInfo(on_wait=[w], on_update=[]),
                        )
                        new_insts.append(nop)
                    ins.sync_info = mybir.SyncInfo(
                        on_wait=keep, on_update=list(si.on_update))
                    changed = True
                    n_fixed += 1
                new_insts.append(ins)
            if changed:
                bb.instructions = new_insts
    return n_fixed


# ---------------- host-side input prep ----------------

def _consts_np():
    ii = np.arange(B)[:, None]
    cc = np.arange(EXT)[None, :]
    c = {}
    c["band"] = ((cc - ii >= 0) & (cc - ii <= 2 * WIN)).astype(np.float32)
    c["pred"] = ((cc - ii >= WIN) & (cc - ii <= 2 * WIN)).astype(np.float32)
    c["suc"] = ((cc - ii >= 0) & (cc - ii <= WIN - 1)).astype(np.float32)
    c["predib"] = ((cc >= ii + WIN) & (cc >= WIN) & (cc < WIN + B)).astype(np.float32)
    c["sucib"] = ((cc < ii + WIN) & (cc >= WIN) & (cc < WIN + B)).astype(np.float32)
    c["diagm"] = (cc == ii + WIN).astype(np.float32)
    cm = {}
    for n, v in c.items():
        cm[n] = np.stack([v[B - WIN:B], v[0:WIN]], axis=1).copy()  # [WIN, 2, EXT]
    return c, cm


def make_in_maps(inputs):
    x = np.asarray(inputs["x"], np.float32)
    spk = np.asarray(inputs["speakers"])
    E = np.zeros((N, NSPK), np.float32)
    E[np.arange(N), spk] = 1.0
    xg = np.zeros((N + 2 * HALO, D), np.float32)
    xg[HALO:HALO + N] = x
    Eg = np.zeros((N + 2 * HALO, NSPK), np.float32)
    Eg[HALO:HALO + N] = E

    bf = ml_dtypes.bfloat16
    w41 = np.concatenate([inputs["W_pred1"], inputs["W_suc1"],
                          inputs["W_same1"], inputs["W_diff1"]], axis=1)
    w42 = np.concatenate([inputs["W_pred2"], inputs["W_suc2"],
                          inputs["W_same2"], inputs["W_diff2"]], axis=1)
    shared = {
        "w41": np.asarray(w41, bf), "w42": np.asarray(w42, bf),
        "wag1": np.asarray(inputs["w_aggr_1"], bf),
        "wag2": np.asarray(inputs["w_aggr_2"], bf),
        "we1": np.asarray(inputs["w_e1"], bf),
        "we2": np.asarray(inputs["w_e2"], bf),
        "ws": np.asarray(inputs["w_s"], bf),
        "be1": np.asarray(inputs["b_e1"], np.float32).reshape(D, 1),
        "be2": np.asarray(inputs["b_e2"], np.float32).reshape(NEMO, 1),
        "bs": np.asarray(inputs["b_s"], np.float32).reshape(NEMO, 1),
    }
    cfull, cmini = _consts_np()
    for n in ("band", "predib", "sucib"):
        shared["c_" + n] = cfull[n]
    for n in ("pred", "suc", "diagm"):
        shared["c8_" + n] = np.tile(
            cfull[n][:, None, :], (1, NBL, 1)).reshape(B, NBL * EXT).copy()
    for n, v in cmini.items():
        shared["cm_" + n] = v

    in_maps = []
    for r in range(CORES):
        lo = r * R
        xc = xg[lo:lo + XR]
        Ec = Eg[lo:lo + XR]
        eTc = np.asarray(Ec.T, bf)
        eOz = np.zeros((NBL, EXT, NSPK1), np.float32)
        for t in range(NBL):
            es = B + B * t
            eOz[t, :, :NSPK] = Ec[es:es + EXT]
            eOz[t, :, NSPK] = Ec[es:es + EXT].sum(axis=1)
            eOz[t, :WIN] = 0.0
            eOz[t, WIN + B:] = 0.0
        eOc = np.asarray(eOz.reshape(NBL * EXT, NSPK1), bf)
        # e4 rows: [+E, +E, -E, -E, 1, 1] -> cross = H0[c]+H1[c]
        #   + (totS - H2[c]) + (totP - H3[c]) in a single contraction
        Ecen = Ec[HALO:HALO + R].T
        e4T = np.concatenate(
            [Ecen, Ecen, -Ecen, -Ecen, np.ones((2, R), np.float32)], axis=0)
        Em = np.concatenate(
            [Ec[B:B + WIN], Ec[HALO + R:HALO + R + WIN]], axis=0).T
        e4Tm = np.concatenate(
            [Em, Em, -Em, -Em, np.ones((2, 2 * WIN), np.float32)], axis=0)
        # local (per-block) triangular weights: cols 0-7 full blocks,
        # col 8 = left mini (gblk r*8-1), col 9 = right mini (gblk (r+1)*8)
        J8 = np.arange(NBL)[:, None]
        T8 = np.arange(NBL)[None, :]
        tls = np.zeros((NBL, 10), np.float32)
        tls[:, :NBL] = (J8 > T8)
        tls[:, 8] = 1.0
        tlp = np.zeros((NBL, 10), np.float32)
        tlp[:, :NBL] = (J8 < T8)
        tlp[:, 9] = 1.0
        triL = np.stack([tls, tlp], axis=1)
        # remote weights over gathered slots [sum, first, last] per core,
        # with edge-block corrections for the mini columns
        trs = np.zeros((3 * CORES, 10), np.float32)
        trp = np.zeros((3 * CORES, 10), np.float32)
        for rr in range(CORES):
            if rr > r:
                trs[3 * rr, :] = 1.0
            if rr < r:
                trp[3 * rr, :] = 1.0
        if r + 1 < CORES:
            trs[3 * (r + 1) + 1, 9] = -1.0
        if r - 1 >= 0:
            trp[3 * (r - 1) + 2, 8] = -1.0
        triR = np.stack([trs, trp], axis=1)
        vm = np.ones((WIN, 2), np.float32)
        if r == 0:
            vm[:, 0] = 0.0
        if r == CORES - 1:
            vm[:, 1] = 0.0
        m = dict(shared)
        m.update({
            "xT": np.ascontiguousarray(xc.T),
            "eT": eTc, "eO": eOc,
            "e4T": np.asarray(e4T, bf), "e4Tm": np.asarray(e4Tm, bf),
            "triL": np.asarray(triL, bf),
            "triR": np.asarray(triR, bf),
            "vmask": vm,
        })
        in_maps.append(m)
    return in_maps


_NC = None


def kernel(**inputs):
    global _NC
    if _NC is None:
        _NC = build_program()
    in_maps = make_in_maps(inputs)
    res = run_bass_kernel_spmd(_NC, in_maps, list(range(CORES)))
    emo = np.concatenate(
        [np.asarray(res.results[r]["emo"]).T for r in range(CORES)], axis=0)
    sen = np.concatenate(
        [np.asarray(res.results[r]["sen"]).T for r in range(CORES)], axis=0)
    return emo, sen

